# revision 13
# baseline (speedup 1.0000x reference)
"""Trainium2 Bass kernel for nn_AFM (attentional factorization machine).

Mathematical reduction (validated against the reference):
  - softmax over a size-1 axis == 1, so the attention MLP is dead code and
    fAtt = mean(fPI, axis=1).
  - FM identity per (b, m): sum_{i<j} x_i x_j = ((sum_i x_i)^2 - sum_i x_i^2)/2
    with x_i = dense[b,i,m] * v[i,m].
  With S1[b,m] = sum_n dense[b,n,m] v[n,m], S2[b,m] = sum_n (dense[b,n,m] v[n,m])^2,
  c[m] = Wp[m] / (2 * P):
    out[b] = sum_n dense[b,n,0] Wl[n] + bl + bp + sum_m c[m] (S1[b,m]^2 - S2[b,m])

Sharding: pure data parallel, batch 4096 -> 512 rows on each of 8 cores.

Raw-bass SPMD program (no Tile framework; manual semaphores) per core:
  GPSIMD: 4 cast-DMA loads (f32->bf16) dispatched up-front; per tile the two
          widest S2-tree add levels.
  DVE:    per tile: dv = d*v (bf16 2x), S1 log-tree, S2 tree tail levels,
          fused combine chain (custom-DVE tensor-tensor-reduce).
  ACT:    per tile: square(dv).
  SYNC:   small param loads up-front; per-tile output stores.
"""

import numpy as np

B, N, M = 4096, 32, 64
NM = N * M                  # 2048
NCORES = 8
BS = B // NCORES            # 512 rows per core
TILES = BS // 128           # 4 tiles of 128 batch rows per core
P_PAIRS = N * (N - 1) // 2  # 496

# tree level widths (outputs): 1024, 512, 256, 128, 64
LVLS = [1024, 512, 256, 128, 64]
GP_LEVELS = 2               # gpsimd takes this many leading S2-tree levels

_CACHE = {}


WAIT_OVERRIDES = {('vch', 16): 17, ('vch', 31): 32, ('vch', 46): 47}  # sim-calibrated


def _build_program():
    import concourse.bass as bass
    from concourse import mybir
    from concourse.dve_ops import TENSOR_TENSOR_REDUCE as CTTR

    f32 = mybir.dt.float32
    bf16 = mybir.dt.bfloat16
    alu = mybir.AluOpType

    from concourse import bacc

    nc = bacc.Bacc("TRN2", target_bir_lowering=False, debug=False)
    dense = nc.declare_dram_parameter("dense", [BS, NM], f32, isOutput=False)
    vrep = nc.declare_dram_parameter("vrep", [128, NM], bf16, isOutput=False)
    crep = nc.declare_dram_parameter("crep", [128, M], f32, isOutput=False)
    wlrep = nc.declare_dram_parameter("wlrep", [128, N], f32, isOutput=False)
    cst = nc.declare_dram_parameter("cst", [128, 1], f32, isOutput=False)
    out = nc.declare_dram_parameter("out", [BS], f32, isOutput=True)

    sb = lambda name, shape, dt: nc.alloc_sbuf_tensor(name, list(shape), dt)

    vrep_t = sb("vrep_t", [128, NM], bf16)
    crep_t = sb("crep_t", [128, M], f32)
    wlrep_t = sb("wlrep_t", [128, N], f32)
    cst_t = sb("cst_t", [128, 1], f32)

    d_t, dv_t, sq_t = [], [], []
    s1lv, s2lv = [], []
    cs1_t, junkM, junkN, pc1_t, pc2_t, o2_t = [], [], [], [], [], []
    for t in range(TILES):
        d_t.append(sb(f"d{t}", [128, NM], bf16))
        dv_t.append(sb(f"dv{t}", [128, NM], bf16))
        sq_t.append(sb(f"sq{t}", [128, NM], bf16))
        s1lv.append(
            [sb(f"s1_{t}_{w}", [128, w], f32 if w == M else bf16) for w in LVLS]
        )
        s2lv.append(
            [sb(f"s2_{t}_{w}", [128, w], f32 if w == M else bf16) for w in LVLS]
        )
        cs1_t.append(sb(f"cs1_{t}", [128, M], f32))
        junkM.append(sb(f"junkM_{t}", [128, M], f32))
        junkN.append(sb(f"junkN_{t}", [128, N], f32))
        pc1_t.append(sb(f"pc1_{t}", [128, 1], f32))
        pc2_t.append(sb(f"pc2_{t}", [128, 1], f32))
        o2_t.append(sb(f"o2_{t}", [128, 1], f32))

    def tree_step(eng, t, which, lvl):
        """One halving add of tree `which` ('s1'/'s2') for tile t, level lvl."""
        levels = s1lv[t] if which == "s1" else s2lv[t]
        src = (dv_t[t] if which == "s1" else sq_t[t]).ap() if lvl == 0 else levels[lvl - 1].ap()
        w = LVLS[lvl]
        return eng.tensor_add(levels[lvl].ap(), src[:, 0:w], src[:, w : 2 * w])

    # Per-engine chain semaphores: every compute instruction waits on its
    # engine chain at the current count and increments it; standalone waits
    # bridge cross-engine dependencies (happens-before propagates through the
    # chains transitively). DMAs keep their own completion semaphores in the
    # single update slot, so they join the chain only on the wait side.
    cnt = {"v": 0, "a": 0, "g": 0, "s": 0}
    chains = {}

    def emit(e, ins):
        ins._wait_ge(chains[e], cnt[e]).then_inc(chains[e], 1)
        cnt[e] += 1
        return cnt[e]

    def emit_dma(e, ins, sem, inc):
        ins._wait_ge(chains[e], cnt[e]).then_inc(sem, inc)

    def emit_wait(e, eng, sem, val):
        val = WAIT_OVERRIDES.get((sem.name if hasattr(sem, "name") else str(sem), val), val)
        eng.wait_ge(sem, val).then_inc(chains[e], 1)
        cnt[e] += 1

    dv_done = [0] * TILES   # vchain value after dv-mul of tile t
    sq_done = [0] * TILES   # achain value after square of tile t
    g2_done = [0] * TILES   # gchain value after gpsimd s2 levels of tile t
    o2_done = [0] * TILES   # vchain value after final combine of tile t

    with (
        nc.Block() as block,
        nc.semaphore("vch") as vch,
        nc.semaphore("ach") as ach,
        nc.semaphore("gch") as gch,
        nc.semaphore("sch") as sch,
        nc.semaphore("ld0") as ld0,
        nc.semaphore("ld1") as ld1,
        nc.semaphore("ld2") as ld2,
        nc.semaphore("ld3") as ld3,
        nc.semaphore("prm") as prm,
        nc.semaphore("sts") as sts,
    ):
        chains.update(v=vch, a=ach, g=gch, s=sch)
        lds = [ld0, ld1, ld2, ld3]

        @block.vector
        def _(dve):
            emit_wait("v", dve, prm, 64)
            for t in range(TILES):
                emit_wait("v", dve, lds[t], 16)
                dv_done[t] = emit(
                    "v", dve.tensor_mul(dv_t[t].ap(), d_t[t].ap(), vrep_t.ap())
                )
                for lvl in range(len(LVLS)):
                    emit("v", tree_step(dve, t, "s1", lvl))
                # s2 tail levels (gpsimd did the first GP_LEVELS)
                emit_wait("v", dve, gch, 3 * (t + 1))
                for lvl in range(GP_LEVELS, len(LVLS)):
                    emit("v", tree_step(dve, t, "s2", lvl))
                # fused combine chain
                emit("v", dve.tensor_mul(cs1_t[t].ap(), s1lv[t][-1].ap(), crep_t.ap()))
                emit("v", dve._custom_dve(
                    CTTR, out=junkM[t].ap(), in0=cs1_t[t].ap(),
                    in1=s1lv[t][-1].ap(), s0=cst_t.ap(), s1=1.0,
                    accum_out=pc1_t[t].ap(),
                ))
                emit("v", dve._custom_dve(
                    CTTR, out=junkM[t].ap(), in0=s2lv[t][-1].ap(),
                    in1=crep_t.ap(), s0=pc1_t[t].ap(), s1=-1.0,
                    accum_out=pc2_t[t].ap(),
                ))
                d_col0 = (
                    d_t[t]
                    .ap()
                    .rearrange("p (n m) -> p n m", n=N)[:, :, 0:1]
                    .rearrange("p n one -> p (n one)")
                )
                o2_done[t] = emit("v", dve._custom_dve(
                    CTTR, out=junkN[t].ap(), in0=d_col0, in1=wlrep_t.ap(),
                    s0=pc2_t[t].ap(), s1=1.0, accum_out=o2_t[t].ap(),
                ))

        @block.scalar
        def _(act):
            for t in range(TILES):
                emit_wait("a", act, vch, dv_done[t])
                sq_done[t] = emit("a", act.square(sq_t[t].ap(), dv_t[t].ap()))

        @block.gpsimd
        def _(gp):
            for t in range(TILES):
                emit_dma(
                    "g",
                    gp.dma_start(
                        out=d_t[t].ap(), in_=dense.ap()[128 * t : 128 * (t + 1), :]
                    ),
                    lds[t], 16,
                )
            for t in range(TILES):
                emit_wait("g", gp, ach, sq_done[t])
                for lvl in range(GP_LEVELS):
                    emit("g", tree_step(gp, t, "s2", lvl))
                g2_done[t] = cnt["g"]
                assert g2_done[t] == 3 * (t + 1)

        @block.sync
        def _(sync):
            emit_dma("s", sync.dma_start(out=vrep_t.ap(), in_=vrep.ap()), prm, 16)
            emit_dma("s", sync.dma_start(out=crep_t.ap(), in_=crep.ap()), prm, 16)
            emit_dma("s", sync.dma_start(out=wlrep_t.ap(), in_=wlrep.ap()), prm, 16)
            emit_dma("s", sync.dma_start(out=cst_t.ap(), in_=cst.ap()), prm, 16)
            for t in range(TILES):
                emit_wait("s", sync, vch, o2_done[t])
                emit_dma(
                    "s",
                    sync.dma_start(
                        out=out.ap()[128 * t : 128 * (t + 1)], in_=o2_t[t].ap()
                    ),
                    sts, 16,
                )
            sync.wait_ge(sts, 16 * TILES)

    nc.compile()
    return nc


def _get_program():
    if "nc" not in _CACHE:
        _CACHE["nc"] = _build_program()
    return _CACHE["nc"]


def _host_prep(inputs):
    import ml_dtypes

    dense = np.ascontiguousarray(
        np.asarray(inputs["dense"], dtype=np.float32).reshape(B, NM)
    )
    v = np.asarray(inputs["v"], dtype=np.float32).reshape(1, NM)
    Wl = np.asarray(inputs["Wl"], dtype=np.float32).reshape(N)
    Wp = np.asarray(inputs["Wp"], dtype=np.float32).reshape(M)
    bl = float(np.asarray(inputs["bl"], dtype=np.float32).reshape(-1)[0])
    bp = float(np.asarray(inputs["bp"], dtype=np.float32).reshape(-1)[0])

    c = (Wp / (2.0 * P_PAIRS)).astype(np.float32)
    vrep = np.ascontiguousarray(
        np.broadcast_to(v.astype(ml_dtypes.bfloat16), (128, NM))
    )
    crep = np.ascontiguousarray(np.broadcast_to(c[None, :], (128, M)))
    wlrep = np.ascontiguousarray(np.broadcast_to(Wl[None, :], (128, N)))
    cst = np.full((128, 1), bl + bp, dtype=np.float32)

    in_maps = []
    for i in range(NCORES):
        in_maps.append(
            {
                "dense": dense[BS * i : BS * (i + 1)],
                "vrep": vrep,
                "crep": crep,
                "wlrep": wlrep,
                "cst": cst,
            }
        )
    return in_maps


def kernel(**inputs) -> np.ndarray:
    from concourse.bass_utils import run_bass_kernel_spmd

    nc = _get_program()
    in_maps = _host_prep(inputs)
    res = run_bass_kernel_spmd(nc, in_maps, core_ids=list(range(NCORES)))
    outs = [np.asarray(res.results[i]["out"], np.float32) for i in range(NCORES)]
    return np.concatenate(outs).reshape(B, 1)
